# revision 4
# baseline (speedup 1.0000x reference)
"""Trainium2 Bass kernel for nn_CustomDistribution (tanh-Gaussian inverse-CDF sampling).

Contract: kernel(mean, std, uniform) takes FULL inputs (4096,16)/(4096,16,1),
shards the 65536 (batch, action) rows across 8 NeuronCores (pure data
parallel), runs a Bass/Tile kernel per core, and returns the full
(sampled_values, sampled_probs) outputs, both (4096, 16) float32.

Math: for each row r with params (mu, sg=std+eps) and grid x_s
(s = 0..1999, linspace(-Y0, Y0)):
    q_s   = c_s * exp(-0.5*((t_s - mu)/sg)^2),  t_s = atanh(x_s), c_s = 1/(1-x_s^2)
    C_s   = cumsum(q),  G = sum(q)
    idx   = #{s : C_s <= u*(G + EPS*sqrt(2*pi)*sg)}    (== reference argmax(u < cdf))
The per-row Gaussian normalizer k = 1/sqrt(2*pi*sg^2) cancels out of the
comparison except through the reference's "+EPS" in the denominator, which is
exactly the EPS/k = EPS*sqrt(2*pi)*sg term above.  idx >= 2000 (only possible
as 2048, all-true) maps to the reference's argmax-of-all-False = 0.
The device returns idx (as f32 count) and G per row; the host gathers
grid[idx] and recomputes the reference's probability formula at idx only.

Grid axis padded 2000 -> 2048 with c_s = 0 so padded q_s == 0 exactly.
"""

import sys

import numpy as np

if "/opt/trn_rl_repo" not in sys.path:
    sys.path.insert(0, "/opt/trn_rl_repo")

EPS = float(np.finfo(np.float32).eps)
S = 2000
SPAD = 2048
Y0 = 0.9999
B, A = 4096, 16
NCORES = 8
ROWS = B * A                      # 65536
ROWS_PER_CORE = ROWS // NCORES    # 8192
TILES = ROWS_PER_CORE // 128      # 64

_CACHE: dict = {}


def _grid_tables():
    """f32 grid tables exactly mirroring the reference's elementwise f32 ops."""
    if "grid" in _CACHE:
        return _CACHE["grid"], _CACHE["t_tab"], _CACHE["c_tab"]
    # Match the reference's jnp.linspace(dtype=float32) bitwise by asking jax
    # itself (on the CPU backend); fall back to a numpy lerp (<=1 ulp off).
    try:
        import jax
        import jax.numpy as jnp

        with jax.default_device(jax.devices("cpu")[0]):
            grid = np.asarray(jnp.linspace(-Y0, Y0, S, dtype=jnp.float32))
    except Exception:
        start, stop = np.float32(-Y0), np.float32(Y0)
        stp = (np.arange(S - 1, dtype=np.float32) / np.float32(S - 1)).astype(
            np.float32
        )
        grid = np.empty(S, np.float32)
        grid[: S - 1] = start * (np.float32(1.0) - stp) + stop * stp
        grid[S - 1] = stop
    one = np.float32(1.0)
    ratio = (one + grid) / (one - grid) + np.float32(EPS)
    t_tab = np.float32(0.5) * np.log(ratio)
    c_tab = one / (one - grid * grid)
    _CACHE["grid"], _CACHE["t_tab"], _CACHE["c_tab"] = grid, t_tab, c_tab
    return grid, t_tab, c_tab


def _build_nc():
    """Build + compile the per-core Bass module (identical on all 8 cores)."""
    if "nc" in _CACHE:
        return _CACHE["nc"]
    import concourse.bass as bass  # noqa: F401
    import concourse.mybir as mybir
    import concourse.tile as tile
    from concourse import bacc

    f32 = mybir.dt.float32
    Af = mybir.ActivationFunctionType
    Op = mybir.AluOpType

    nc = bacc.Bacc(
        "TRN2",
        target_bir_lowering=False,
        debug=False,
        enable_asserts=False,
        num_devices=NCORES,
    )

    t_d = nc.dram_tensor("t_bc", [128, SPAD], f32, kind="ExternalInput").ap()
    c_d = nc.dram_tensor("c_bc", [128, SPAD], f32, kind="ExternalInput").ap()
    negmu_d = nc.dram_tensor("negmu", [128, TILES], f32, kind="ExternalInput").ap()
    sc_d = nc.dram_tensor("sc", [128, TILES], f32, kind="ExternalInput").ap()
    aeps_d = nc.dram_tensor("aeps", [128, TILES], f32, kind="ExternalInput").ap()
    u_d = nc.dram_tensor("uu", [128, TILES], f32, kind="ExternalInput").ap()
    cnt_d = nc.dram_tensor("cnt", [128, TILES], f32, kind="ExternalOutput").ap()
    g_d = nc.dram_tensor("gsum", [128, TILES], f32, kind="ExternalOutput").ap()

    with tile.TileContext(nc) as tc:
        with (
            tc.tile_pool(name="const", bufs=1) as constp,
            tc.tile_pool(name="sq", bufs=3) as sqp,
            tc.tile_pool(name="e", bufs=3) as ep,
            tc.tile_pool(name="q", bufs=3) as qp,
            tc.tile_pool(name="cdf", bufs=3) as cdfp,
            tc.tile_pool(name="msk", bufs=2) as mskp,
        ):
            t_sb = constp.tile([128, SPAD], f32, tag="t_sb")
            nc.sync.dma_start(t_sb[:], t_d)
            c_sb = constp.tile([128, SPAD], f32, tag="c_sb")
            nc.sync.dma_start(c_sb[:], c_d)
            z_sb = constp.tile([128, SPAD], f32, tag="z_sb")
            nc.vector.memset(z_sb[:], 0.0)

            negmu_sb = constp.tile([128, TILES], f32, tag="negmu_sb")
            nc.sync.dma_start(negmu_sb[:], negmu_d)
            sc_sb = constp.tile([128, TILES], f32, tag="sc_sb")
            nc.sync.dma_start(sc_sb[:], sc_d)
            aeps_sb = constp.tile([128, TILES], f32, tag="aeps_sb")
            nc.sync.dma_start(aeps_sb[:], aeps_d)
            u_sb = constp.tile([128, TILES], f32, tag="u_sb")
            nc.sync.dma_start(u_sb[:], u_d)

            cnt_sb = constp.tile([128, TILES], f32, tag="cnt_sb")
            g_sb = constp.tile([128, TILES], f32, tag="g_sb")
            w_sb = constp.tile([128, TILES], f32, tag="w_sb")

            for j in range(TILES):
                jc = slice(j, j + 1)
                # sq = (t - mu)^2   (subtract first: cancellation-safe)
                sq = sqp.tile([128, SPAD], f32)
                nc.scalar.activation(
                    sq[:], t_sb[:], Af.Square, bias=negmu_sb[:, jc], scale=1.0
                )
                # e = exp(sq * (-0.5/sg^2))
                e = ep.tile([128, SPAD], f32)
                nc.scalar.activation(
                    e[:], sq[:], Af.Exp, bias=0.0, scale=sc_sb[:, jc]
                )
                # q = e * c   (tensor_tensor_reduce crashes the PJRT path, so
                # plain multiply; G comes from the cumsum's last column)
                q = qp.tile([128, SPAD], f32)
                nc.vector.tensor_mul(q[:], e[:], c_sb[:])
                # C = cumsum(q)
                C = cdfp.tile([128, SPAD], f32)
                nc.vector.tensor_tensor_scan(
                    C[:], z_sb[:], q[:], 0.0, op0=Op.add, op1=Op.add
                )
                # G = C[:, -1]  (exported for the host-side normalizer)
                nc.scalar.copy(g_sb[:, jc], C[:, SPAD - 1 : SPAD])
                # w = (G + aeps) * u
                nc.vector.tensor_scalar(
                    w_sb[:, jc],
                    C[:, SPAD - 1 : SPAD],
                    aeps_sb[:, jc],
                    u_sb[:, jc],
                    op0=Op.add,
                    op1=Op.mult,
                )
                # cnt = sum(C <= w)
                msk = mskp.tile([128, SPAD], f32)
                nc.vector.tensor_scalar(
                    msk[:],
                    C[:],
                    w_sb[:, jc],
                    None,
                    op0=Op.is_le,
                    op1=Op.add,
                    accum_out=cnt_sb[:, jc],
                )

            nc.sync.dma_start(cnt_d, cnt_sb[:])
            nc.sync.dma_start(g_d, g_sb[:])

    nc.compile()
    _CACHE["nc"] = nc
    return nc


def _to_core_layout(x_flat: np.ndarray, core: int) -> np.ndarray:
    """[ROWS] -> this core's [128, TILES]: row r = j*128 + p  ->  [p, j]."""
    seg = x_flat[core * ROWS_PER_CORE : (core + 1) * ROWS_PER_CORE]
    return np.ascontiguousarray(seg.reshape(TILES, 128).T)


def _from_core_layout(mats: list) -> np.ndarray:
    """Inverse of _to_core_layout over all cores -> [ROWS]."""
    return np.concatenate([np.asarray(m).T.reshape(-1) for m in mats])


def kernel(mean, std, uniform):
    from concourse.bass_utils import run_bass_kernel_spmd

    mean = np.asarray(mean, dtype=np.float32)
    std = np.asarray(std, dtype=np.float32)
    uniform = np.asarray(uniform, dtype=np.float32)

    grid, t_tab, c_tab = _grid_tables()
    nc = _build_nc()

    mu = mean.reshape(ROWS)
    sg = std.reshape(ROWS) + np.float32(EPS)
    u = uniform.reshape(ROWS)

    sg64 = sg.astype(np.float64)
    negmu = (-mu).astype(np.float32)
    sc = (-0.5 / (sg64 * sg64)).astype(np.float32)
    aeps = (EPS * np.sqrt(2.0 * np.pi) * sg64).astype(np.float32)

    t_pad = np.zeros(SPAD, np.float32)
    t_pad[:S] = t_tab
    c_pad = np.zeros(SPAD, np.float32)
    c_pad[:S] = c_tab
    t_bc = np.ascontiguousarray(np.broadcast_to(t_pad, (128, SPAD)))
    c_bc = np.ascontiguousarray(np.broadcast_to(c_pad, (128, SPAD)))

    in_maps = [
        {
            "t_bc": t_bc,
            "c_bc": c_bc,
            "negmu": _to_core_layout(negmu, c),
            "sc": _to_core_layout(sc, c),
            "aeps": _to_core_layout(aeps, c),
            "uu": _to_core_layout(u, c),
        }
        for c in range(NCORES)
    ]

    trace = bool(_CACHE.get("trace", False))
    res = run_bass_kernel_spmd(
        nc, in_maps, core_ids=list(range(NCORES)), trace=trace
    )
    if trace:
        _CACHE["exec_time_ns"] = res.exec_time_ns
        _CACHE["profile_json"] = res.profile_json
        _CACHE["trace_result"] = res
    cnt = _from_core_layout([r["cnt"] for r in res.results])
    G = _from_core_layout([r["gsum"] for r in res.results])

    idx = cnt.astype(np.int64)
    idx[idx >= S] = 0

    # Host gather + reference-exact f32 probability at the sampled index only.
    vals = grid[idx]
    t_i = t_tab[idx]
    c_i = c_tab[idx]
    diff = t_i - mu
    log_term = (diff * diff) / (np.float32(-2.0) * (sg * sg))
    pk = np.float32(1.0) / np.sqrt(np.float32(2.0 * np.pi) * (sg * sg))
    p_unnorm = c_i * pk * np.exp(log_term)
    denom = pk * G.astype(np.float32) + np.float32(EPS)
    probs = p_unnorm / denom

    return vals.reshape(B, A), probs.reshape(B, A)


# revision 6
# speedup vs baseline: 1.1759x; 1.1759x over previous
"""Trainium2 Bass kernel for nn_CustomDistribution (tanh-Gaussian inverse-CDF sampling).

Contract: kernel(mean, std, uniform) takes FULL inputs (4096,16)/(4096,16,1),
shards the 65536 (batch, action) rows across 8 NeuronCores (pure data
parallel), runs a Bass/Tile kernel per core, and returns the full
(sampled_values, sampled_probs) outputs, both (4096, 16) float32.

Math: for each row r with params (mu, sg=std+eps) and grid x_s
(s = 0..1999, linspace(-Y0, Y0)):
    q_s   = c_s * exp(-0.5*((t_s - mu)/sg)^2),  t_s = atanh(x_s), c_s = 1/(1-x_s^2)
    C_s   = cumsum(q),  G = sum(q)
    idx   = #{s : C_s <= u*(G + EPS*sqrt(2*pi)*sg)}    (== reference argmax(u < cdf))
The per-row Gaussian normalizer k = 1/sqrt(2*pi*sg^2) cancels out of the
comparison except through the reference's "+EPS" in the denominator, which is
exactly the EPS/k = EPS*sqrt(2*pi)*sg term above.  idx >= 2000 (only possible
as 2048, all-true) maps to the reference's argmax-of-all-False = 0.
The device returns idx (as f32 count) and G per row; the host gathers
grid[idx] and recomputes the reference's probability formula at idx only.

Grid axis padded 2000 -> 2048 with c_s = 0 so padded q_s == 0 exactly.
"""

import sys

import numpy as np

if "/opt/trn_rl_repo" not in sys.path:
    sys.path.insert(0, "/opt/trn_rl_repo")

EPS = float(np.finfo(np.float32).eps)
S = 2000
SPAD = 2048
Y0 = 0.9999
B, A = 4096, 16
NCORES = 8
ROWS = B * A                      # 65536
ROWS_PER_CORE = ROWS // NCORES    # 8192
TILES = ROWS_PER_CORE // 128      # 64
ALPHA = 1.0e20  # tanh saturation scale for the ACT-side compare+count

_CACHE: dict = {}


def _grid_tables():
    """f32 grid tables exactly mirroring the reference's elementwise f32 ops."""
    if "grid" in _CACHE:
        return _CACHE["grid"], _CACHE["t_tab"], _CACHE["c_tab"]
    # Match the reference's jnp.linspace(dtype=float32) bitwise by asking jax
    # itself (on the CPU backend); fall back to a numpy lerp (<=1 ulp off).
    try:
        import jax
        import jax.numpy as jnp

        with jax.default_device(jax.devices("cpu")[0]):
            grid = np.asarray(jnp.linspace(-Y0, Y0, S, dtype=jnp.float32))
    except Exception:
        start, stop = np.float32(-Y0), np.float32(Y0)
        stp = (np.arange(S - 1, dtype=np.float32) / np.float32(S - 1)).astype(
            np.float32
        )
        grid = np.empty(S, np.float32)
        grid[: S - 1] = start * (np.float32(1.0) - stp) + stop * stp
        grid[S - 1] = stop
    one = np.float32(1.0)
    ratio = (one + grid) / (one - grid) + np.float32(EPS)
    t_tab = np.float32(0.5) * np.log(ratio)
    c_tab = one / (one - grid * grid)
    _CACHE["grid"], _CACHE["t_tab"], _CACHE["c_tab"] = grid, t_tab, c_tab
    return grid, t_tab, c_tab


def _build_nc():
    """Build + compile the per-core Bass module (identical on all 8 cores)."""
    if "nc" in _CACHE:
        return _CACHE["nc"]
    import concourse.bass as bass  # noqa: F401
    import concourse.mybir as mybir
    import concourse.tile as tile
    from concourse import bacc

    f32 = mybir.dt.float32
    Af = mybir.ActivationFunctionType
    Op = mybir.AluOpType

    nc = bacc.Bacc(
        "TRN2",
        target_bir_lowering=False,
        debug=False,
        enable_asserts=False,
        num_devices=NCORES,
    )

    t_d = nc.dram_tensor("t_bc", [128, SPAD], f32, kind="ExternalInput").ap()
    c_d = nc.dram_tensor("c_bc", [128, SPAD], f32, kind="ExternalInput").ap()
    negmu_d = nc.dram_tensor("negmu", [128, TILES], f32, kind="ExternalInput").ap()
    sc_d = nc.dram_tensor("sc", [128, TILES], f32, kind="ExternalInput").ap()
    aeps_d = nc.dram_tensor("aeps", [128, TILES], f32, kind="ExternalInput").ap()
    u_d = nc.dram_tensor("uu", [128, TILES], f32, kind="ExternalInput").ap()
    cnt_d = nc.dram_tensor("cnt", [128, TILES], f32, kind="ExternalOutput").ap()
    g_d = nc.dram_tensor("gsum", [128, TILES], f32, kind="ExternalOutput").ap()

    with tile.TileContext(nc) as tc:
        with (
            tc.tile_pool(name="const", bufs=1) as constp,
            tc.tile_pool(name="sq", bufs=3) as sqp,
            tc.tile_pool(name="e", bufs=3) as ep,
            tc.tile_pool(name="q", bufs=3) as qp,
            tc.tile_pool(name="cdf", bufs=3) as cdfp,
            tc.tile_pool(name="msk", bufs=2) as mskp,
        ):
            t_sb = constp.tile([128, SPAD], f32, tag="t_sb")
            nc.sync.dma_start(t_sb[:], t_d)
            c_sb = constp.tile([128, SPAD], f32, tag="c_sb")
            nc.sync.dma_start(c_sb[:], c_d)
            z_sb = constp.tile([128, SPAD], f32, tag="z_sb")
            nc.vector.memset(z_sb[:], 0.0)

            negmu_sb = constp.tile([128, TILES], f32, tag="negmu_sb")
            nc.sync.dma_start(negmu_sb[:], negmu_d)
            sc_sb = constp.tile([128, TILES], f32, tag="sc_sb")
            nc.sync.dma_start(sc_sb[:], sc_d)
            aeps_sb = constp.tile([128, TILES], f32, tag="aeps_sb")
            nc.sync.dma_start(aeps_sb[:], aeps_d)
            u_sb = constp.tile([128, TILES], f32, tag="u_sb")
            nc.sync.dma_start(u_sb[:], u_d)

            cnt_sb = constp.tile([128, TILES], f32, tag="cnt_sb")
            g_sb = constp.tile([128, TILES], f32, tag="g_sb")
            w_sb = constp.tile([128, TILES], f32, tag="w_sb")

            for j in range(TILES):
                jc = slice(j, j + 1)
                # sq = (t - mu)^2   (subtract first: cancellation-safe)
                sq = sqp.tile([128, SPAD], f32)
                nc.scalar.activation(
                    sq[:], t_sb[:], Af.Square, bias=negmu_sb[:, jc], scale=1.0
                )
                # e = exp(sq * (-0.5/sg^2))
                e = ep.tile([128, SPAD], f32)
                nc.scalar.activation(
                    e[:], sq[:], Af.Exp, bias=0.0, scale=sc_sb[:, jc]
                )
                # q = e * c   (tensor_tensor_reduce crashes the PJRT path, so
                # plain multiply; G comes from the cumsum's last column)
                q = qp.tile([128, SPAD], f32)
                nc.vector.tensor_mul(q[:], e[:], c_sb[:])
                # C = cumsum(q)
                C = cdfp.tile([128, SPAD], f32)
                nc.vector.tensor_tensor_scan(
                    C[:], z_sb[:], q[:], 0.0, op0=Op.add, op1=Op.add
                )
                # G = C[:, -1]  (exported for the host-side normalizer)
                nc.scalar.copy(g_sb[:, jc], C[:, SPAD - 1 : SPAD])
                # w' = (G + aeps) * u'   with u' = ALPHA*(1+2^-22)*u folded in
                # on the host, so w' = ALPHA * w (slightly upward-perturbed).
                nc.vector.tensor_scalar(
                    w_sb[:, jc],
                    C[:, SPAD - 1 : SPAD],
                    aeps_sb[:, jc],
                    u_sb[:, jc],
                    op0=Op.add,
                    op1=Op.mult,
                )
                # count on ACT (DVE is the bottleneck): acc = sum_s tanh(
                # ALPHA*(w - C_s)) = #below - #above, saturated to +-1.
                # Host recovers idx = (acc + SPAD)/2.  Tanh shares the
                # exp_and_others LUT set with Exp -> no table reloads.
                msk = mskp.tile([128, SPAD], f32)
                nc.scalar.activation(
                    msk[:],
                    C[:],
                    Af.Tanh,
                    bias=w_sb[:, jc],
                    scale=-ALPHA,
                    accum_out=cnt_sb[:, jc],
                )

            nc.sync.dma_start(cnt_d, cnt_sb[:])
            nc.sync.dma_start(g_d, g_sb[:])

    nc.compile()
    _CACHE["nc"] = nc
    return nc


def _to_core_layout(x_flat: np.ndarray, core: int) -> np.ndarray:
    """[ROWS] -> this core's [128, TILES]: row r = j*128 + p  ->  [p, j]."""
    seg = x_flat[core * ROWS_PER_CORE : (core + 1) * ROWS_PER_CORE]
    return np.ascontiguousarray(seg.reshape(TILES, 128).T)


def _from_core_layout(mats: list) -> np.ndarray:
    """Inverse of _to_core_layout over all cores -> [ROWS]."""
    return np.concatenate([np.asarray(m).T.reshape(-1) for m in mats])


def kernel(mean, std, uniform):
    from concourse.bass_utils import run_bass_kernel_spmd

    mean = np.asarray(mean, dtype=np.float32)
    std = np.asarray(std, dtype=np.float32)
    uniform = np.asarray(uniform, dtype=np.float32)

    grid, t_tab, c_tab = _grid_tables()
    nc = _build_nc()

    mu = mean.reshape(ROWS)
    sg = std.reshape(ROWS) + np.float32(EPS)
    u = uniform.reshape(ROWS)

    sg64 = sg.astype(np.float64)
    negmu = (-mu).astype(np.float32)
    u_dev = (u.astype(np.float64) * (ALPHA * (1.0 + 2.0**-22))).astype(np.float32)
    sc = (-0.5 / (sg64 * sg64)).astype(np.float32)
    aeps = (EPS * np.sqrt(2.0 * np.pi) * sg64).astype(np.float32)

    t_pad = np.zeros(SPAD, np.float32)
    t_pad[:S] = t_tab
    c_pad = np.zeros(SPAD, np.float32)
    c_pad[:S] = c_tab
    t_bc = np.ascontiguousarray(np.broadcast_to(t_pad, (128, SPAD)))
    c_bc = np.ascontiguousarray(np.broadcast_to(c_pad, (128, SPAD)))

    in_maps = [
        {
            "t_bc": t_bc,
            "c_bc": c_bc,
            "negmu": _to_core_layout(negmu, c),
            "sc": _to_core_layout(sc, c),
            "aeps": _to_core_layout(aeps, c),
            "uu": _to_core_layout(u_dev, c),
        }
        for c in range(NCORES)
    ]

    trace = bool(_CACHE.get("trace", False))
    res = run_bass_kernel_spmd(
        nc, in_maps, core_ids=list(range(NCORES)), trace=trace
    )
    if trace:
        _CACHE["exec_time_ns"] = res.exec_time_ns
        _CACHE["profile_json"] = res.profile_json
        _CACHE["trace_result"] = res
    cnt = _from_core_layout([r["cnt"] for r in res.results])
    G = _from_core_layout([r["gsum"] for r in res.results])

    idx = np.floor((cnt + SPAD) * 0.5 + 0.5).astype(np.int64)
    idx[idx >= S] = 0

    # Host gather + reference-exact f32 probability at the sampled index only.
    vals = grid[idx]
    t_i = t_tab[idx]
    c_i = c_tab[idx]
    diff = t_i - mu
    log_term = (diff * diff) / (np.float32(-2.0) * (sg * sg))
    pk = np.float32(1.0) / np.sqrt(np.float32(2.0 * np.pi) * (sg * sg))
    p_unnorm = c_i * pk * np.exp(log_term)
    denom = pk * G.astype(np.float32) + np.float32(EPS)
    probs = p_unnorm / denom

    return vals.reshape(B, A), probs.reshape(B, A)
